# revision 8
# baseline (speedup 1.0000x reference)
"""Trainium2 Bass kernel for a custom RNN layer.

Math (per batch row n):
    xp[t] = x[t] @ Wx + bx                      (big parallel GEMM)
    h[t]  = tanh(xp[t] + h[t-1] @ Wh + bh)      (strictly sequential in t)
    output = all h[t], shape (N, T, D_H)

Strategy (8 NeuronCores, data-parallel over batch N=64 -> 8 rows/core):
  * Everything on-chip runs in a TRANSPOSED layout: hidden state h^T with the
    feature dim on SBUF partitions (2 halves of 128) and the batch (8) on the
    free dim.  This keeps the per-step tanh a single cheap ACT instruction and
    makes the recurrence matmuls self-consistent (no transposes anywhere).
  * seq_x is pre-transposed on the host (pure layout marshalling) so every DMA
    is contiguous; the output is produced transposed and untransposed on host.
  * Phase A (input projection) is done per 64-step window directly INTO the
    PSUM banks the recurrence will accumulate into:
        psum[jh][j, 8t+n] = sum_i Wx[i, 128jh+j] * x[n, t, i]
    then a DVE broadcast-add folds (bx + bh) into the same banks.  The
    recurrence then only adds Wh-blocks times h^T per step (4 small matmuls)
    and one Tanh ACT per step finishes the step.
"""

import numpy as np

import concourse.bass as bass
import concourse.tile as tile
from concourse import bacc, mybir
from concourse.bass import ds
from concourse.bass_utils import run_bass_kernel_spmd

F32 = mybir.dt.float32
F32R = mybir.dt.float32r
F16 = mybir.dt.float16

N, T, D = 64, 2048, 256          # full problem
NCORES = 8
NB = N // NCORES                 # batch rows per core = 8
W = 64                           # recurrence steps per psum window
NW = T // W                      # 32 windows
TANH = mybir.ActivationFunctionType.Tanh

# Phase-A (input projection) matmul dtype.  float32r is fp32 data in the PE's
# fast mode: 1 cycle/row at moving-dim >= 256 and a single LDWEIGHTS+MATMUL
# pass (plain fp32 lowers to TWO half-speed passes).
PHASE_A_DT = F32R
# Recurrence dtype for Wh and the hidden state.  fp16 keeps the weight load
# on the fast path (FWL) and a single matmul pass; |h| <= 1 and |Wh| ~ N(0,1)
# are comfortably in fp16 range, and the fp16 rounding (~5e-4) is far below
# the fp32-reordering chaos floor of this recurrence.
REC_DT = F16
OUT_NP_DT = np.float16


def build_program(T=T):
    NW = T // W
    nc = bacc.Bacc(
        "TRN2",
        target_bir_lowering=False,
        debug=False,
        enable_asserts=False,
        num_devices=NCORES,
    )

    # Per-core DRAM tensors.  xt / ht / out_t are host-side transposed layouts.
    xt = nc.dram_tensor("xt", [D, T, NB], PHASE_A_DT, kind="ExternalInput").ap()
    wx = nc.dram_tensor("wx", [D, D], PHASE_A_DT, kind="ExternalInput").ap()
    wh = nc.dram_tensor("wh", [D, D], F32, kind="ExternalInput").ap()
    bxb = nc.dram_tensor("bx", [1, D], F32, kind="ExternalInput").ap()
    bhb = nc.dram_tensor("bh", [1, D], F32, kind="ExternalInput").ap()
    ht0 = nc.dram_tensor("ht0", [D, NB], F32, kind="ExternalInput").ap()
    out_t = nc.dram_tensor("out_t", [D, T, NB], REC_DT, kind="ExternalOutput").ap()

    with tile.TileContext(nc) as tc:
        with (
            tc.tile_pool(name="singles", bufs=1) as singles,
            tc.tile_pool(name="xtp", bufs=2) as xtp,
            tc.tile_pool(name="stage", bufs=2) as stagep,
            tc.tile_pool(name="psum", bufs=2, space="PSUM") as psump,
        ):
            # ---- constants ------------------------------------------------
            # w*_sb[i, kh, j] = W*[128*kh + i, j]
            wx_sb = singles.tile([128, 2, D], PHASE_A_DT)
            nc.sync.dma_start(out=wx_sb, in_=wx.rearrange("(h i) j -> i h j", h=2))
            wh_f32 = singles.tile([128, 2, D], F32)
            nc.sync.dma_start(out=wh_f32, in_=wh.rearrange("(h i) j -> i h j", h=2))
            wh_sb = singles.tile([128, 2, D], REC_DT)
            nc.vector.tensor_copy(wh_sb, wh_f32)

            # bias_sb[j, jh] = bx[128*jh + j] + bh[128*jh + j]
            bx_sb = singles.tile([128, 2], F32)
            nc.sync.dma_start(out=bx_sb, in_=bxb.rearrange("o (h j) -> j (o h)", h=2))
            bh_sb = singles.tile([128, 2], F32)
            nc.sync.dma_start(out=bh_sb, in_=bhb.rearrange("o (h j) -> j (o h)", h=2))
            bias_sb = singles.tile([128, 2], F32)
            nc.vector.tensor_add(bias_sb, bx_sb, bh_sb)

            # hT0[i, kh, n] = h_init[n, 128*kh + i]
            ht0_f32 = singles.tile([128, 2, NB], F32)
            nc.sync.dma_start(out=ht0_f32, in_=ht0.rearrange("(h i) n -> i h n", h=2))
            ht0_sb = singles.tile([128, 2, NB], REC_DT)
            nc.vector.tensor_copy(ht0_sb, ht0_f32)

            prev = None  # (h^T k-half 0 AP, k-half 1 AP) of the previous step

            for w in range(NW):
                t0 = w * W
                # x^T window: xw[i, kh, t, n] = x[n, t0+t, 128*kh+i]
                xw = xtp.tile([128, 2, W, NB], PHASE_A_DT, tag="xw")
                nc.sync.dma_start(
                    out=xw,
                    in_=xt[:, t0 : t0 + W, :].rearrange("(h i) t n -> i h t n", h=2),
                )
                # two psum banks for this window: psum2[j, jh, 8t+n]
                psum2 = psump.tile([128, 2, W * NB], F32, tag="ps")
                # stage2[j, jh, 8t+n] accumulates this window's h^T outputs
                stage2 = stagep.tile([128, 2, W * NB], REC_DT, tag="st")

                # ---- phase A: xp for the whole window straight into PSUM --
                for jh in range(2):
                    for kh in range(2):
                        nc.tensor.matmul(
                            psum2[:, jh, :],
                            lhsT=wx_sb[:, kh, ds(128 * jh, 128)],
                            rhs=xw[:, kh, :, :],
                            start=(kh == 0),
                            stop=(kh == 1),
                        )
                    # fold bx + bh into the bank
                    nc.vector.tensor_scalar_add(
                        psum2[:, jh, :], psum2[:, jh, :], bias_sb[:, ds(jh, 1)]
                    )

                # ---- recurrence over the window ---------------------------
                for twi in range(W):
                    if prev is None:
                        rhs = (ht0_sb[:, 0, :], ht0_sb[:, 1, :])
                    else:
                        rhs = prev
                    for jh in range(2):
                        for kh in range(2):
                            nc.tensor.matmul(
                                psum2[:, jh, ds(NB * twi, NB)],
                                lhsT=wh_sb[:, kh, ds(128 * jh, 128)],
                                rhs=rhs[kh],
                                start=False,
                                stop=(kh == 1),
                                skip_group_check=True,
                            )
                    nc.scalar.activation(
                        out=stage2[:, :, ds(NB * twi, NB)],
                        in_=psum2[:, :, ds(NB * twi, NB)],
                        func=TANH,
                    )
                    prev = (
                        stage2[:, 0, ds(NB * twi, NB)],
                        stage2[:, 1, ds(NB * twi, NB)],
                    )

                # ---- write the window's h^T back --------------------------
                nc.sync.dma_start(
                    out=out_t[:, t0 : t0 + W, :].rearrange("(h j) t n -> j h t n", h=2),
                    in_=stage2.rearrange("j h (t n) -> j h t n", n=NB),
                )

    nc.compile()
    return nc


_NC_CACHE = None


def _get_program():
    global _NC_CACHE
    if _NC_CACHE is None:
        _NC_CACHE = build_program()
    return _NC_CACHE


def kernel(seq_x, Wx, bx, Wh, bh, h_init):
    seq_x = np.ascontiguousarray(np.asarray(seq_x, dtype=np.float32))
    Wx = np.ascontiguousarray(np.asarray(Wx, dtype=np.float32))
    bx = np.ascontiguousarray(np.asarray(bx, dtype=np.float32))
    Wh = np.ascontiguousarray(np.asarray(Wh, dtype=np.float32))
    bh = np.ascontiguousarray(np.asarray(bh, dtype=np.float32))
    h_init = np.ascontiguousarray(np.asarray(h_init, dtype=np.float32))

    nc = _get_program()

    in_maps = []
    for c in range(NCORES):
        sl = slice(c * NB, (c + 1) * NB)
        in_maps.append(
            {
                # xt[i, t, n] = x[n, t, i]
                "xt": np.ascontiguousarray(seq_x[sl].transpose(2, 1, 0)),
                "wx": Wx,
                "wh": Wh,
                "bx": bx,
                "bh": bh,
                # ht0[i, n] = h_init[n, i]
                "ht0": np.ascontiguousarray(h_init[sl].T),
            }
        )

    res = run_bass_kernel_spmd(nc, in_maps, core_ids=list(range(NCORES)))

    outs = []
    for c in range(NCORES):
        ot = res.results[c]["out_t"].astype(np.float32)  # [D, T, NB]
        outs.append(ot.transpose(2, 1, 0))  # -> [NB, T, D]
    return np.ascontiguousarray(np.concatenate(outs, axis=0), dtype=np.float32)


if __name__ == "__main__":
    rng = np.random.default_rng(0)
    inputs = {
        "seq_x": rng.standard_normal((N, T, D), dtype=np.float32),
        "Wx": rng.standard_normal((D, D), dtype=np.float32),
        "bx": rng.standard_normal((1, D), dtype=np.float32),
        "Wh": rng.standard_normal((D, D), dtype=np.float32),
        "bh": rng.standard_normal((1, D), dtype=np.float32),
        "h_init": rng.standard_normal((N, D), dtype=np.float32),
    }
    out = kernel(**inputs)
    print(out.shape, out.dtype)
